# revision 26
# baseline (speedup 1.0000x reference)
"""Multi-head attention (B=4, S=2048, D=1024, H=16, Dk=64) on 8 trn2 NeuronCores.

Sharding: core = (batch b, head-group g), b in 0..3, g in 0..1.  Each core
computes attention for its batch and its 8 heads plus the partial out
projection for its 512 rows of Wo; host sums the two partials per batch and
adds bo.

Key optimizations over the naive version:
  - Host-side key compaction: mask keys (~50% zeros) are gathered out of k/v
    before upload, so the device only scores/exps/attends over valid keys
    (padded to a multiple of 128; pad lanes get a -1e9 exp bias -> probs 0).
  - Host-side transposes + bf16 casts: q/k/v arrive as [D, S] bf16, so phase A
    needs no PE transposes, weight loads use FWL, and DMA bytes halve.
  - Scores for a head PAIR run as two concurrent row-tiled K=64 matmuls
    (partition bases 0/64) into one PSUM tile, so a single [128,1024] ACT exp
    covers both heads (ACT is the bottleneck engine; fewer+wider ACTIVATEs).
  - The ones-column in vh yields softmax denominators for free (row 64 of the
    attn PSUM); reciprocal via the fast approx DVE op; recip broadcast across
    64 partitions with a K=1 matmul; normalize straight out of PSUM.
  - Software-pipelined emission (engine queues are FIFO): scores(sk+1) is
    emitted before attn(sk); normalization of the previous pair and the out
    projection of the previous query block are emitted as PE fillers early in
    the next pair's key loop.
"""

import sys

sys.path.insert(0, "/opt/trn_rl_repo")

import numpy as np

B, S, D, H, DK = 4, 2048, 1024, 16, 64
CPG = 512          # projection columns per core (8 heads x 64)
NPAIR = 4          # head pairs per core
NDCH = D // 128    # contraction chunks for projections
NCORES = 8

_cache = {}


def _build_nc(nskv, zero_bias):
    import contextlib

    import concourse.bass as bass
    import concourse.tile as tile
    from concourse import bacc, mybir

    f32 = mybir.dt.float32
    bf16 = mybir.dt.bfloat16
    Exp = mybir.ActivationFunctionType.Exp

    skv = nskv * 128

    nc = bacc.Bacc("TRN2", target_bir_lowering=False, debug=False)

    qt_d = nc.dram_tensor("qt", [D, S], bf16, kind="ExternalInput").ap()
    kt_d = nc.dram_tensor("kt", [D, skv], bf16, kind="ExternalInput").ap()
    vt_d = nc.dram_tensor("vt", [D, skv], bf16, kind="ExternalInput").ap()
    wq_d = nc.dram_tensor("wq", [D, CPG], bf16, kind="ExternalInput").ap()
    wk_d = nc.dram_tensor("wk", [D, CPG], bf16, kind="ExternalInput").ap()
    wv_d = nc.dram_tensor("wv", [D, CPG], bf16, kind="ExternalInput").ap()
    wo_d = nc.dram_tensor("wo", [CPG, D], bf16, kind="ExternalInput").ap()
    mb_d = nc.dram_tensor("maskbias", [128, nskv], f32, kind="ExternalInput").ap()
    ones_d = nc.dram_tensor("ones", [128, 512], bf16, kind="ExternalInput").ap()
    if not zero_bias:
        bq_d = nc.dram_tensor("bq", [128, NPAIR], f32, kind="ExternalInput").ap()
        bk_d = nc.dram_tensor("bk", [128, NPAIR], f32, kind="ExternalInput").ap()
        bv_d = nc.dram_tensor("bv", [1, CPG], bf16, kind="ExternalInput").ap()
    out_d = nc.dram_tensor("out", [S, D], f32, kind="ExternalOutput").ap()

    # skv split into 512-wide column chunks for the khT projection copies
    kq_chunks = []
    o = 0
    while o < skv:
        w = min(512, skv - o)
        kq_chunks.append((o, w))
        o += w

    with tile.TileContext(nc) as tc:
        with contextlib.ExitStack() as ctx:
            # ---------- persistent tensors + constants ----------
            persist = ctx.enter_context(tc.tile_pool(name="persist", bufs=1))
            consts = ctx.enter_context(tc.tile_pool(name="consts", bufs=1))

            qhT_sb = persist.tile([128, NPAIR, S], bf16)      # [c%128, pair, sq]
            khT_sb = persist.tile([128, NPAIR, skv], bf16)    # [c%128, pair, sk]
            vh_sb = persist.tile([128, nskv, 8, 128], bf16)  # [vdims|ones|zeros]
            concatT_sb = persist.tile([128, NPAIR, S], bf16)

            ones_sb = consts.tile([1, 512], bf16)
            nc.gpsimd.memset(ones_sb, 1.0)
            swdge_warm = consts.tile([1, 16], bf16)
            nc.gpsimd.dma_start(out=swdge_warm, in_=ones_sb[0:1, 0:16])
            lib_warm = consts.tile([2, 16], f32)
            nc.gpsimd.memset(lib_warm[0:1, :], 1.0)
            nc.gpsimd.partition_broadcast(lib_warm, lib_warm[0:1, :])
            nc.gpsimd.memset(vh_sb, 0.0)
            nc.gpsimd.memset(vh_sb[:, :, :, DK], 1.0)
            mb_sb = consts.tile([128, nskv], f32)
            wo_sb = consts.tile([128, NPAIR, D], bf16)

            def emit_const_dmas():
                nc.sync.dma_start(out=mb_sb, in_=mb_d)
                for j in range(NPAIR):
                    nc.sync.dma_start(
                        out=wo_sb[:, j, :], in_=wo_d[j * 128 : j * 128 + 128, :]
                    )

            if not zero_bias:
                bq_sb = consts.tile([128, NPAIR], f32)
                nc.sync.dma_start(out=bq_sb, in_=bq_d)
                bk_sb = consts.tile([128, NPAIR], f32)
                nc.sync.dma_start(out=bk_sb, in_=bk_d)
                bv_sb = consts.tile([1, CPG], bf16)
                nc.sync.dma_start(out=bv_sb, in_=bv_d)

            # ---------- phase A: projections (no transposes needed) ----------
            xpool = ctx.enter_context(tc.tile_pool(name="xpool", bufs=1))
            wpool = ctx.enter_context(tc.tile_pool(name="wpool", bufs=1))
            with contextlib.ExitStack() as actx:
                prpool = actx.enter_context(
                    tc.tile_pool(name="prpool", bufs=8, space="PSUM")
                )

                # HAM warmup: the PE clock-gate needs ~3.4us of sustained
                # activity to reach full clock; burn the initial DMA wait on
                # dummy matmuls over constant data so phase A starts warm
                warm_ps = prpool.tile([128, 512], f32, tag="pr", name="warmps")
                for w in range(16):
                    nc.tensor.matmul(
                        warm_ps,
                        lhsT=ones_sb[0:1, 0:128],
                        rhs=ones_sb[0:1, :],
                        start=(w == 0),
                        stop=(w == 15),
                    )

                # k projection: khT[c, sk] = Wk^T @ kT
                kt_sb = xpool.tile([128, NDCH, skv], bf16, tag="kt")
                wk_sb = wpool.tile([128, NDCH, CPG], bf16, tag="wk")
                for j in range(NDCH):
                    nc.sync.dma_start(out=wk_sb[:, j, :], in_=wk_d[j * 128 : j * 128 + 128, :])
                    if j < 2:
                        h = skv // 2
                        nc.scalar.dma_start(out=kt_sb[:, j, 0:h], in_=kt_d[j * 128 : j * 128 + 128, 0:h])
                        nc.scalar.dma_start(out=kt_sb[:, j, h:skv], in_=kt_d[j * 128 : j * 128 + 128, h:skv])
                    else:
                        nc.scalar.dma_start(out=kt_sb[:, j, :], in_=kt_d[j * 128 : j * 128 + 128, :])
                emit_const_dmas()
                for o, w in kq_chunks:
                    prs = [prpool.tile([128, 512], f32, tag="pr", name=f"pr{i}") for i in range(4)]
                    for j in range(NDCH):
                        for cch in range(4):
                            nc.tensor.matmul(
                                prs[cch][:, 0:w],
                                lhsT=wk_sb[:, j, cch * 128 : cch * 128 + 128],
                                rhs=kt_sb[:, j, o : o + w],
                                start=(j == 0),
                                stop=(j == NDCH - 1),
                            )
                    for cch in range(4):
                        if zero_bias:
                            nc.vector.tensor_copy(
                                out=khT_sb[:, cch, o : o + w], in_=prs[cch][:, 0:w]
                            )
                        else:
                            nc.vector.tensor_scalar_add(
                                khT_sb[:, cch, o : o + w],
                                prs[cch][:, 0:w],
                                bk_sb[:, cch : cch + 1],
                            )

                # v projection: vh[sk, c] = vT^T @ Wv  (per 128-key chunk)
                vt_sb = xpool.tile([128, NDCH, skv], bf16, tag="vt")
                wv_sb = wpool.tile([128, NDCH, CPG], bf16, tag="wv")
                for j in range(NDCH):
                    nc.scalar.dma_start(out=vt_sb[:, j, :], in_=vt_d[j * 128 : j * 128 + 128, :])
                    nc.sync.dma_start(out=wv_sb[:, j, :], in_=wv_d[j * 128 : j * 128 + 128, :])
                for skc in range(nskv):
                    pr = prpool.tile([128, 512], f32, tag="pr")
                    for j in range(NDCH):
                        if not zero_bias and j == 0:
                            nc.tensor.matmul(
                                pr,
                                lhsT=ones_sb[0:1, 0:128],
                                rhs=bv_sb[0:1, :],
                                start=True,
                                stop=False,
                            )
                        nc.tensor.matmul(
                            pr,
                            lhsT=vt_sb[:, j, skc * 128 : skc * 128 + 128],
                            rhs=wv_sb[:, j, :],
                            start=(zero_bias and j == 0),
                            stop=(j == NDCH - 1),
                        )
                    nc.vector.tensor_copy(
                        out=vh_sb[:, skc, :, 0:DK],
                        in_=pr.rearrange("p (h d) -> p h d", h=8),
                    )

                # q projection: qhT[c, sq] = Wq^T @ qT
                qt_sb = xpool.tile([128, NDCH, S], bf16, tag="qt")
                wq_sb = wpool.tile([128, NDCH, CPG], bf16, tag="wq")
                for j in range(NDCH):
                    nc.scalar.dma_start(out=qt_sb[:, j, :], in_=qt_d[j * 128 : j * 128 + 128, :])
                    nc.sync.dma_start(out=wq_sb[:, j, :], in_=wq_d[j * 128 : j * 128 + 128, :])
                for sqq in range(1):
                    o = sqq * 512
                    prs = [prpool.tile([128, 512], f32, tag="pr", name=f"pr{i}") for i in range(4)]
                    for j in range(NDCH):
                        for cch in range(4):
                            nc.tensor.matmul(
                                prs[cch],
                                lhsT=wq_sb[:, j, cch * 128 : cch * 128 + 128],
                                rhs=qt_sb[:, j, o : o + 512],
                                start=(j == 0),
                                stop=(j == NDCH - 1),
                            )
                    for cch in range(4):
                        if zero_bias:
                            nc.vector.tensor_copy(
                                out=qhT_sb[:, cch, o : o + 512], in_=prs[cch]
                            )
                        else:
                            nc.vector.tensor_scalar_add(
                                qhT_sb[:, cch, o : o + 512],
                                prs[cch],
                                bq_sb[:, cch : cch + 1],
                            )

            # ---------- phase B + C: attention, fused with out projection ----
            with contextlib.ExitStack() as bctx:
                probpool = bctx.enter_context(tc.tile_pool(name="probpool", bufs=4))
                dnpool = bctx.enter_context(tc.tile_pool(name="dnpool", bufs=3))
                rc32pool = bctx.enter_context(tc.tile_pool(name="rc32pool", bufs=3))
                rc16pool = bctx.enter_context(tc.tile_pool(name="rc16pool", bufs=2))
                outpool = bctx.enter_context(tc.tile_pool(name="outpool", bufs=3))
                scpool = bctx.enter_context(
                    tc.tile_pool(name="scpool", bufs=2, space="PSUM")
                )
                atpool = bctx.enter_context(
                    tc.tile_pool(name="atpool", bufs=2, space="PSUM")
                )
                auxpool = bctx.enter_context(
                    tc.tile_pool(name="auxpool", bufs=2, space="PSUM")
                )

                def emit_norm_fast(sqb, pair, atA, atB):
                    """Low-latency variant for the final pair (tail): direct
                    [1,512] reciprocal instead of the DMA-reshape round trip."""
                    q0 = sqb * 512
                    for hh, at in ((0, atA), (1, atB)):
                        base = hh * 64
                        atf = rc32pool.tile([128, 512], f32, tag="atf")
                        nc.vector.tensor_copy(out=atf[0:65, :], in_=at[0:65, :])
                        r1 = dnpool.tile([1, 512], f32, tag="rfull")
                        nc.vector.reciprocal(r1, atf[64:65, :])
                        rep = rc16pool.tile([64, 512], f32, tag="rep")
                        nc.gpsimd.partition_broadcast(rep, r1)
                        nc.vector.tensor_mul(
                            concatT_sb[base : base + 64, pair, q0 : q0 + 512],
                            atf[0:64, :],
                            rep,
                        )

                def emit_norm(sqb, pair, atA, atB):
                    """Normalize both heads of a finished pair into concatT.

                    The PSUM->SBUF copy comes first so the attn PSUM slot is
                    released ~0.7us after the last attn matmul.  The [1,512]
                    denominator row is reshaped to [4,128] by DMA so the DVE
                    reciprocal runs lane-parallel (0.8us instead of 3.3us),
                    DMA'd back to a row, broadcast across 64 partitions and
                    multiplied on the otherwise-idle GPSIMD engine -- off
                    every critical queue."""
                    q0 = sqb * 512
                    heads = ((0, atA), (1, atB))
                    atfs, r4s, reps = {}, {}, {}
                    for hh, at in heads:
                        atf = rc32pool.tile([128, 512], f32, tag="atf")
                        nc.vector.tensor_copy(out=atf[0:65, :], in_=at[0:65, :])
                        atfs[hh] = atf
                    for hh, at in heads:
                        dn4 = dnpool.tile([4, 128], f32, tag="dn4")
                        nc.gpsimd.dma_start(out=dn4, in_=atfs[hh][64:65, :])
                        r4 = dnpool.tile([4, 128], f32, tag="r4")
                        nc.vector.reciprocal(r4, dn4)
                        r4s[hh] = r4
                    for hh, at in heads:
                        rfull = dnpool.tile([1, 512], f32, tag="rfull")
                        nc.gpsimd.dma_start(out=rfull, in_=r4s[hh])
                        rep = rc16pool.tile([64, 512], f32, tag="rep")
                        nc.gpsimd.partition_broadcast(rep, rfull)
                        reps[hh] = rep
                    for hh, at in heads:
                        base = hh * 64
                        nc.vector.tensor_mul(
                            concatT_sb[base : base + 64, pair, q0 : q0 + 512],
                            atfs[hh][0:64, :],
                            reps[hh],
                        )

                def emit_outproj(sqb, sqc):
                    """One 128-query chunk of the out projection + store."""
                    q0 = sqb * 512 + sqc * 128
                    for do in range(2):
                        ops = auxpool.tile([128, 512], f32, tag="aux")
                        for j in range(NPAIR):
                            nc.tensor.matmul(
                                ops,
                                lhsT=concatT_sb[:, j, q0 : q0 + 128],
                                rhs=wo_sb[:, j, do * 512 : do * 512 + 512],
                                start=(j == 0),
                                stop=(j == NPAIR - 1),
                            )
                        osb = outpool.tile([128, 512], f32, tag="osb")
                        nc.vector.tensor_copy(out=osb, in_=ops)
                        nc.sync.dma_start(
                            out=out_d[q0 : q0 + 128, do * 512 : do * 512 + 512],
                            in_=osb,
                        )

                def emit_qproj(sqq, cch):
                    """One deferred q-projection chunk (PE filler in phase B)."""
                    o = sqq * 512
                    pr = auxpool.tile([128, 512], f32, tag="aux")
                    for j in range(NDCH):
                        nc.tensor.matmul(
                            pr,
                            lhsT=wq_sb[:, j, cch * 128 : cch * 128 + 128],
                            rhs=qt_sb[:, j, o : o + 512],
                            start=(j == 0),
                            stop=(j == NDCH - 1),
                        )
                    if zero_bias:
                        nc.vector.tensor_copy(out=qhT_sb[:, cch, o : o + 512], in_=pr)
                    else:
                        nc.vector.tensor_scalar_add(
                            qhT_sb[:, cch, o : o + 512], pr, bq_sb[:, cch : cch + 1]
                        )

                # fillers are (closure, ) lists emitted inside the NEXT pair's
                # sk loop, after a couple of score/exp stages are in flight;
                # pending2 fires later in the same pair (sk==5)
                pending = []
                pending2 = []

                for sqb in range(4):
                    for pair in range(NPAIR):
                        hA, hB = 2 * pair, 2 * pair + 1
                        q0 = sqb * 512
                        atA = atpool.tile([128, 512], f32, tag="at")
                        atB = atpool.tile([128, 512], f32, tag="at")

                        sc_tiles = [None] * nskv
                        probs_tiles = [None] * nskv

                        def emit_scores_exp(sk):
                            sc = scpool.tile([128, 1024], f32, tag="sc")
                            sc_tiles[sk] = sc
                            nc.tensor.matmul(
                                sc[:, 0:512],
                                lhsT=khT_sb[0:64, pair, sk * 128 : sk * 128 + 128],
                                rhs=qhT_sb[0:64, pair, q0 : q0 + 512],
                                start=True,
                                stop=True,
                            )
                            nc.tensor.matmul(
                                sc[:, 512:1024],
                                lhsT=khT_sb[64:128, pair, sk * 128 : sk * 128 + 128],
                                rhs=qhT_sb[64:128, pair, q0 : q0 + 512],
                                start=True,
                                stop=True,
                            )
                            probs = probpool.tile([128, 1024], bf16, tag="probs")
                            probs_tiles[sk] = probs
                            nc.scalar.activation(
                                out=probs,
                                in_=sc,
                                func=Exp,
                                bias=mb_sb[:, sk : sk + 1],
                                scale=0.125,
                            )

                        def emit_attn(sk):
                            probs = probs_tiles[sk]
                            nc.tensor.matmul(
                                atA,
                                lhsT=vh_sb[:, sk, hA, :],
                                rhs=probs[:, 0:512],
                                start=(sk == 0),
                                stop=(sk == nskv - 1),
                            )
                            nc.tensor.matmul(
                                atB,
                                lhsT=vh_sb[:, sk, hB, :],
                                rhs=probs[:, 512:1024],
                                start=(sk == 0),
                                stop=(sk == nskv - 1),
                            )

                        # software pipeline: scores(sk) runs one stage ahead
                        # of attn(sk); pending fillers (prev pair's norm, prev
                        # sqb's out-proj chunk) drop in after stage 2's scores
                        emit_scores_exp(0)
                        for sk in range(1, nskv):
                            emit_scores_exp(sk)
                            if sk == 2:
                                for f in pending:
                                    f()
                                pending = []
                            if sk == 5:
                                for f in pending2:
                                    f()
                                pending2 = []
                            emit_attn(sk - 1)
                        emit_attn(nskv - 1)

                        if sqb == 3 and pair == NPAIR - 1:
                            last_atA, last_atB = atA, atB
                        else:
                            pending.append(
                                lambda sqb=sqb, pair=pair, atA=atA, atB=atB: emit_norm(
                                    sqb, pair, atA, atB
                                )
                            )
                        if sqb > 0:
                            pending.append(
                                lambda sqb=sqb, pair=pair: emit_outproj(
                                    sqb - 1, pair
                                )
                            )
                        if sqb < 3:
                            pending2.append(
                                lambda sqq=sqb + 1, cch=pair: emit_qproj(sqq, cch)
                            )

                # drain: last pair's norm (low-latency variant) + sqb 3's
                # out projection
                for f in pending:
                    f()
                emit_norm(3, 3, last_atA, last_atB)
                # keep the PE warm through the final norm chain so the last
                # out-projection runs at full clock (HAM would re-throttle
                # after ~3.4us of idle): a few dummy matmuls into a retired
                # scores slot, spaced by the scheduler's natural dependencies
                for w in range(4):
                    warm = scpool.tile([128, 1024], f32, tag="sc", name=f"warm{w}")
                    nc.tensor.matmul(
                        warm[:, 0:512],
                        lhsT=khT_sb[0:64, 0, 0:128],
                        rhs=qhT_sb[0:64, 0, 0:512],
                        start=True,
                        stop=True,
                    )
                for sqc in range(4):
                    emit_outproj(3, sqc)

    nc.compile()
    return nc


def get_nc(nskv=9, zero_bias=True):
    key = (nskv, zero_bias)
    if key not in _cache:
        _cache[key] = _build_nc(nskv, zero_bias)
    return _cache[key]


def make_in_maps(q, k, v, mask, Wq, bq, Wk, bk, Wv, bv, Wo, bo):
    import ml_dtypes

    f32 = np.float32
    bf16 = ml_dtypes.bfloat16
    c = np.ascontiguousarray

    mask = np.asarray(mask)
    idxs = [np.nonzero(mask[b, 0] != 0)[0] for b in range(B)]
    kvs = [len(ix) for ix in idxs]
    nskv = max(1, (max(kvs) + 127) // 128)
    skv = nskv * 128

    zero_bias = (
        not np.any(np.asarray(bq))
        and not np.any(np.asarray(bk))
        and not np.any(np.asarray(bv))
    )

    Wq, Wk, Wv, Wo = (np.asarray(a, f32) for a in (Wq, Wk, Wv, Wo))

    in_maps = []
    for core in range(NCORES):
        b, g = core // 2, core % 2
        cols = slice(g * CPG, (g + 1) * CPG)
        ix = idxs[b]
        kv = kvs[b]

        kc = np.zeros((skv, D), f32)
        vc = np.zeros((skv, D), f32)
        kc[:kv] = np.asarray(k[b], f32)[ix]
        vc[:kv] = np.asarray(v[b], f32)[ix]

        mbflat = np.where(np.arange(skv) < kv, 0.0, -1e9).astype(f32)

        m = {
            "qt": c(np.asarray(q[b], f32).T.astype(bf16)),
            "kt": c(kc.T.astype(bf16)),
            "vt": c(vc.T.astype(bf16)),
            "wq": c(Wq[:, cols].astype(bf16)),
            "wk": c(Wk[:, cols].astype(bf16)),
            "wv": c(Wv[:, cols].astype(bf16)),
            "wo": c(Wo[cols, :].astype(bf16)),
            "maskbias": c(mbflat.reshape(nskv, 128).T),
            "ones": np.ones((128, 512), bf16),
        }
        if not zero_bias:
            m["bq"] = c(np.asarray(bq, f32)[cols].reshape(NPAIR, 128).T)
            m["bk"] = c(np.asarray(bk, f32)[cols].reshape(NPAIR, 128).T)
            m["bv"] = c(np.asarray(bv, f32)[cols].reshape(1, CPG).astype(bf16))
        in_maps.append(m)
    return in_maps, nskv, zero_bias


def gather(results, bo):
    out = np.zeros((B, S, D), np.float32)
    for core in range(NCORES):
        b = core // 2
        out[b] += results[core]["out"]
    out += np.asarray(bo, np.float32)[None, None, :]
    return out


def run_on_hw(in_maps, nskv, zero_bias, trace=False, trace_cores=None):
    from concourse.bass_utils import run_bass_kernel_spmd

    nc = get_nc(nskv, zero_bias)
    return run_bass_kernel_spmd(
        nc,
        in_maps,
        list(range(NCORES)),
        trace=trace,
        trace_cores=trace_cores,
    )


def kernel(q, k, v, mask, Wq, bq, Wk, bk, Wv, bv, Wo, bo):
    in_maps, nskv, zero_bias = make_in_maps(
        q, k, v, mask, Wq, bq, Wk, bk, Wv, bv, Wo, bo
    )
    res = run_on_hw(in_maps, nskv, zero_bias)
    return gather(res.results, bo)
